# revision 7
# baseline (speedup 1.0000x reference)
"""Trainium2 Bass kernel for CombinedHiddenGCVAE (GCN conditional VAE).

Graph: N=50000 nodes, E=800000 edges. Five GCN propagations (enc_i2h,
enc_mean+enc_logvar fused, dec_i2h, dec_out) over the same normalized
adjacency D^-1/2 (A+I) D^-1/2.

Distribution: nodes sharded 8 ways (6250/core, padded to 49 blocks of 128).
Each conv k: every core builds its shard of the "gather table"
xw_k[n] = dinv[n] * (h_{k-1} @ W_k)[n], AllGathers it into a full
[50176, 64] DRAM table, then processes its own dst-edges: dma_gather the
src rows, segmented-sum by dst via one-hot S-matrix matmuls accumulated in
PSUM per 128-dst block, scale columns by dinv[dst], bias/activation, and
immediately compute + ship the next conv's table rows.
"""
import hashlib
import numpy as np

N = 50000
E = 800000
FEAT, COND, HID, LAT = 64, 32, 64, 32
NCORES = 8
SH = 6250                  # real nodes per core
BLK = 128
NB = 49                    # blocks per core (49*128 = 6272)
NSH = NB * BLK             # padded shard rows
QCH = 7                    # blocks per AllGather chunk (7 chunks of 7)
NQ = NB // QCH             # 7 AG chunks
ROWS_Q = NCORES * QCH * BLK   # 7168 table rows per AG chunk
V = NSH * NCORES           # 50176 table rows
HALFV = V // 2             # 25088 (int16 gather split)
SG = 4                     # blocks per gather supergroup
NSG = (NB + SG - 1) // SG  # 13
SCHUNK = 8                 # tiles per S-build op

_cache = {}


def _trow(n):
    """node id -> table row (AllGather chunk-major layout)."""
    n = np.asarray(n)
    c, r = np.divmod(n, SH)
    b, p = np.divmod(r, BLK)
    q, bq = np.divmod(b, QCH)
    return q * ROWS_Q + c * (QCH * BLK) + bq * BLK + p


def _wrap16_cols(idx_tile):
    """[128] idx of one tile -> [128, 8] int16 wrapped (16-partition, replicated)."""
    a = idx_tile.astype(np.int16).reshape(8, 16).T        # [16, 8]
    return np.tile(a, (8, 1))                             # [128, 8]


def _prep_edges(src, dst):
    """Host-side per-core edge layout. Returns per-core tensors + shared tile
    counts."""
    deg = np.bincount(dst, minlength=N).astype(np.float32) + 1.0
    dinv = (1.0 / np.sqrt(deg)).astype(np.float32)

    per_core = []
    for c in range(NCORES):
        lo_n, hi_n = c * SH, (c + 1) * SH
        m = (dst >= lo_n) & (dst < hi_n)
        s = np.concatenate([src[m], np.arange(lo_n, hi_n, dtype=np.int64)])
        dl = np.concatenate([dst[m] - lo_n, np.arange(SH, dtype=np.int64)])
        tr = _trow(s)
        b = dl // BLK
        dm = (dl % BLK).astype(np.float32)
        hi = tr >= HALFV
        per_core.append((tr, b, dm, hi))

    # shared tile counts per (block, half)
    T_LO = np.zeros(NB, np.int64)
    T_HI = np.zeros(NB, np.int64)
    for (tr, b, dm, hi) in per_core:
        for half, T in ((False, T_LO), (True, T_HI)):
            sel = b[hi == half]
            cnt = np.bincount(sel, minlength=NB)
            T = np.maximum(T, (cnt + BLK - 1) // BLK, out=T)
    T_LO = np.maximum(T_LO, 1)
    T_HI = np.maximum(T_HI, 1)

    TT_LO, TT_HI = int(T_LO.sum()), int(T_HI.sum())

    # build padded per-core streams
    cores = []
    for (tr, b, dm, hi) in per_core:
        idx_cols = {"lo": np.zeros((128, TT_LO * 8), np.int16),
                    "hi": np.zeros((128, TT_HI * 8), np.int16)}
        dmod = {"lo": np.full((128, TT_LO), 999.0, np.float32),
                "hi": np.full((128, TT_HI), 999.0, np.float32)}
        off = {"lo": 0, "hi": 0}
        for blk in range(NB):
            for half, key, T in ((False, "lo", T_LO), (True, "hi", T_HI)):
                sel = (b == blk) & (hi == half)
                rows = tr[sel] - (HALFV if half else 0)
                dms = dm[sel]
                ntile = int(T[blk])
                pad = ntile * BLK - len(rows)
                rows = np.concatenate([rows, np.zeros(pad, np.int64)])
                dmsp = np.concatenate([dms, np.full(pad, 999.0, np.float32)])
                o = off[key]
                for t in range(ntile):
                    idx_cols[key][:, (o + t) * 8:(o + t + 1) * 8] = \
                        _wrap16_cols(rows[t * BLK:(t + 1) * BLK])
                    dmod[key][:, o + t] = dmsp[t * BLK:(t + 1) * BLK]
                off[key] += ntile
        cores.append((idx_cols["lo"], idx_cols["hi"], dmod["lo"], dmod["hi"]))
    return dinv, T_LO, T_HI, cores


def _build_program(T_LO, T_HI):
    import concourse.bass as bass
    import concourse.bacc as bacc
    import concourse.mybir as mybir
    import concourse.tile as tile

    dt = mybir.dt
    TT_LO, TT_HI = int(T_LO.sum()), int(T_HI.sum())
    # supergroup tile spans per stream
    sg_blocks = [list(range(s * SG, min((s + 1) * SG, NB))) for s in range(NSG)]
    LO_OFF = np.concatenate([[0], np.cumsum(T_LO)]).astype(int)
    HI_OFF = np.concatenate([[0], np.cumsum(T_HI)]).astype(int)
    MAXSG_LO = max(int(T_LO[bs[0]:bs[-1] + 1].sum()) for bs in sg_blocks)
    MAXSG_HI = max(int(T_HI[bs[0]:bs[-1] + 1].sum()) for bs in sg_blocks)
    MAXT = int(max(T_LO.max(), T_HI.max()))

    nc = bacc.Bacc("TRN2", target_bir_lowering=False, debug=False,
                   num_devices=NCORES, dynamic_dma_scratch_size=49152)

    # ---- I/O ----
    xT_in = nc.dram_tensor("xT_in", [FEAT + COND, NSH], dt.float32,
                           kind="ExternalInput")
    condT_in = nc.dram_tensor("condT_in", [64, NSH], dt.float32,
                              kind="ExternalInput")   # rows 32:64 = cond^T
    noiseT_in = nc.dram_tensor("noiseT_in", [LAT, NSH], dt.float32,
                               kind="ExternalInput")
    dinvsb_in = nc.dram_tensor("dinvsb_in", [128, NB], dt.float32,
                               kind="ExternalInput")
    dinvrep_in = nc.dram_tensor("dinvrep_in", [64, NSH], dt.float32,
                                kind="ExternalInput")
    iota_in = nc.dram_tensor("iota_in", [128, 128], dt.float32,
                             kind="ExternalInput")
    idxlo_in = nc.dram_tensor("idxlo_in", [128, TT_LO * 8], dt.int16,
                              kind="ExternalInput")
    idxhi_in = nc.dram_tensor("idxhi_in", [128, TT_HI * 8], dt.int16,
                              kind="ExternalInput")
    dmlo_in = nc.dram_tensor("dmlo_in", [128, TT_LO], dt.float32,
                             kind="ExternalInput")
    dmhi_in = nc.dram_tensor("dmhi_in", [128, TT_HI], dt.float32,
                             kind="ExternalInput")
    w1_in = nc.dram_tensor("w1_in", [FEAT + COND, HID], dt.float32,
                           kind="ExternalInput")
    w23_in = nc.dram_tensor("w23_in", [HID, 64], dt.float32, kind="ExternalInput")
    w4_in = nc.dram_tensor("w4_in", [64, HID], dt.float32, kind="ExternalInput")
    w5_in = nc.dram_tensor("w5_in", [HID, FEAT], dt.float32, kind="ExternalInput")
    b1_in = nc.dram_tensor("b1_in", [64, 1], dt.float32, kind="ExternalInput")
    b23_in = nc.dram_tensor("b23_in", [64, 1], dt.float32, kind="ExternalInput")
    b4_in = nc.dram_tensor("b4_in", [64, 1], dt.float32, kind="ExternalInput")
    b5_in = nc.dram_tensor("b5_in", [64, 1], dt.float32, kind="ExternalInput")

    zT_out = nc.dram_tensor("zT_out", [NB, LAT, BLK], dt.float32,
                            kind="ExternalOutput")
    meanT_out = nc.dram_tensor("meanT_out", [NB, LAT, BLK], dt.float32,
                               kind="ExternalOutput")
    logvarT_out = nc.dram_tensor("logvarT_out", [NB, LAT, BLK], dt.float32,
                                 kind="ExternalOutput")
    outT_out = nc.dram_tensor("outT_out", [NB, FEAT, BLK], dt.float32,
                              kind="ExternalOutput")

    # ---- collective buffers ----
    tables = {}
    agins = {}
    for k in (1, 2, 4, 5):
        tables[k] = nc.dram_tensor(f"table{k}", [V, 64], dt.float32,
                                   kind="Internal", addr_space="Shared")
        agins[k] = [nc.dram_tensor(f"agin{k}_{q}", [QCH * BLK, 64], dt.float32,
                                   kind="Internal") for q in range(NQ)]

    with tile.TileContext(nc) as tc:
        with (
            tc.tile_pool(name="const", bufs=1) as constp,
            tc.tile_pool(name="stlo", bufs=2) as stlop,
            tc.tile_pool(name="sthi", bufs=2) as sthip,
            tc.tile_pool(name="sbuf", bufs=4) as sbufp,
            tc.tile_pool(name="smat", bufs=4) as smatp,
            tc.tile_pool(name="psA", bufs=3, space="PSUM") as psA,
            tc.tile_pool(name="psB", bufs=3, space="PSUM") as psB,
        ):
            # ---- constants to SBUF ----
            xT = constp.tile([FEAT + COND, NSH], dt.float32)
            zcT = constp.tile([64, NSH], dt.float32)
            noiseT = constp.tile([LAT, NSH], dt.float32)
            dinvsb = constp.tile([128, NB], dt.float32)
            dinvrep = constp.tile([64, NSH], dt.float32)
            iota = constp.tile([128, 128], dt.float32)
            idxlo = constp.tile([128, TT_LO * 8], dt.int16)
            idxhi = constp.tile([128, TT_HI * 8], dt.int16)
            dmlo = constp.tile([128, TT_LO], dt.float32)
            dmhi = constp.tile([128, TT_HI], dt.float32)
            w1 = constp.tile([FEAT + COND, HID], dt.float32)
            w23 = constp.tile([HID, 64], dt.float32)
            w4 = constp.tile([64, HID], dt.float32)
            w5 = constp.tile([HID, FEAT], dt.float32)
            b1 = constp.tile([64, 1], dt.float32)
            b23 = constp.tile([64, 1], dt.float32)
            b4 = constp.tile([64, 1], dt.float32)
            b5 = constp.tile([64, 1], dt.float32)
            for t, src_ in ((xT, xT_in), (zcT, condT_in), (noiseT, noiseT_in),
                            (dinvsb, dinvsb_in), (dinvrep, dinvrep_in),
                            (iota, iota_in), (idxlo, idxlo_in), (idxhi, idxhi_in),
                            (dmlo, dmlo_in), (dmhi, dmhi_in),
                            (w1, w1_in), (w23, w23_in), (w4, w4_in), (w5, w5_in),
                            (b1, b1_in), (b23, b23_in), (b4, b4_in), (b5, b5_in)):
                nc.sync.dma_start(out=t[:], in_=src_[:])

            def ship_table(k, b, rowsT_sbuf):
                """rowsT_sbuf [128,64] = table rows for block b (already
                dinv-scaled); DMA to agin, fire AG chunk when complete."""
                q, bq = divmod(b, QCH)
                nc.sync.dma_start(
                    out=agins[k][q][bq * BLK:(bq + 1) * BLK, :],
                    in_=rowsT_sbuf[:])
                if bq == QCH - 1:
                    nc.gpsimd.collective_compute(
                        "AllGather", mybir.AluOpType.bypass,
                        replica_groups=[list(range(NCORES))],
                        ins=[agins[k][q][:]],
                        outs=[tables[k][q * ROWS_Q:(q + 1) * ROWS_Q, :]],
                    )

            # ---- phase 1: build table1 = dinv * (x @ W1) ----
            for b in range(NB):
                xw = psB.tile([128, 64], dt.float32, tag="xwn")
                nc.tensor.matmul(out=xw[:], lhsT=xT[:, b * BLK:(b + 1) * BLK],
                                 rhs=w1[:], start=True, stop=True)
                xws = sbufp.tile([128, 64], dt.float32, tag="xws")
                nc.scalar.activation(out=xws[:], in_=xw[:],
                                     func=mybir.ActivationFunctionType.Copy,
                                     scale=dinvsb[:, b:b + 1])
                ship_table(1, b, xws)

            # ---- propagation convs ----
            GT = 8   # tiles per gather instruction (1024 descriptors)

            def propagate(k, postproc):
                """gather from tables[k], segmented-sum, call postproc(b, aggd)
                with aggd = [64,128] SBUF tile of dinv-scaled aggregation."""
                stage_tiles = {"lo": {}, "hi": {}}

                def get_stage(key, g):
                    if g in stage_tiles[key]:
                        return stage_tiles[key][g]
                    TT = TT_LO if key == "lo" else TT_HI
                    idxs = idxlo if key == "lo" else idxhi
                    pool = stlop if key == "lo" else sthip
                    src = tables[k][:] if key == "lo" else tables[k][HALFV:, :]
                    L = min(GT, TT - g * GT)
                    st = pool.tile([128, L, 64], dt.float32, tag="g" + key,
                                   padded_shape=[128, GT, 64])
                    nc.gpsimd.dma_gather(
                        out_ap=st[:], in_ap=src,
                        idxs_ap=idxs[:, g * GT * 8:(g * GT + L) * 8],
                        num_idxs=L * BLK, num_idxs_reg=L * BLK, elem_size=64)
                    stage_tiles[key][g] = st
                    return st

                for b in range(NB):
                    agg = psA.tile([64, 128], dt.float32, tag="agg")
                    ntl, nth = int(T_LO[b]), int(T_HI[b])
                    first = True
                    for key, nt, base, dm in (
                            ("lo", ntl, int(LO_OFF[b]), dmlo),
                            ("hi", nth, int(HI_OFF[b]), dmhi)):
                        sm = smatp.tile([128, MAXT * 128], dt.float32,
                                        tag="sm")
                        nc.vector.tensor_tensor(
                            out=sm[:, :nt * 128],
                            in0=dm[:, base:base + nt]
                                .to_broadcast([128, nt, 128]),
                            in1=iota[:, None, :].to_broadcast([128, nt, 128]),
                            op=mybir.AluOpType.is_equal,
                        )
                        for t in range(nt):
                            gi, go = divmod(base + t, GT)
                            stage = get_stage(key, gi)
                            last = (key == "hi") and (t == nth - 1)
                            nc.tensor.matmul(
                                out=agg[:],
                                lhsT=stage[:, go, :],
                                rhs=sm[:, t * 128:(t + 1) * 128],
                                start=first, stop=last)
                            first = False
                    aggd = sbufp.tile([64, 128], dt.float32, tag="aggd")
                    nc.vector.tensor_mul(
                        out=aggd[:], in0=agg[:],
                        in1=dinvrep[:, b * BLK:(b + 1) * BLK])
                    postproc(b, aggd)

            AF = None  # set below

            def post1(b, aggd):
                hT = sbufp.tile([64, 128], dt.float32, tag="hT")
                nc.scalar.activation(out=hT[:], in_=aggd[:], func=AF.Relu,
                                     bias=b1[:])
                xw = psB.tile([128, 64], dt.float32, tag="xwn")
                nc.tensor.matmul(out=xw[:], lhsT=hT[:], rhs=w23[:],
                                 start=True, stop=True)
                xws = sbufp.tile([128, 64], dt.float32, tag="xws")
                nc.scalar.activation(out=xws[:], in_=xw[:], func=AF.Copy,
                                     scale=dinvsb[:, b:b + 1])
                ship_table(2, b, xws)

            def post23(b, aggd):
                cols = slice(b * BLK, (b + 1) * BLK)
                meanT = sbufp.tile([LAT, 128], dt.float32, tag="meanT")
                logvT = sbufp.tile([LAT, 128], dt.float32, tag="logvT")
                nc.scalar.activation(out=meanT[:], in_=aggd[:LAT, :],
                                     func=AF.Identity, bias=b23[:LAT, :])
                nc.scalar.activation(out=logvT[:], in_=aggd[LAT:, :],
                                     func=AF.Identity, bias=b23[LAT:, :])
                expT = sbufp.tile([LAT, 128], dt.float32, tag="expT")
                nc.scalar.activation(out=expT[:], in_=logvT[:], func=AF.Exp,
                                     scale=0.5)
                # z = noise * exp + mean, written into zcT rows 0:32
                nc.vector.tensor_mul(out=expT[:], in0=expT[:],
                                     in1=noiseT[:, cols])
                nc.vector.tensor_add(out=zcT[:LAT, cols], in0=expT[:],
                                     in1=meanT[:])
                nc.sync.dma_start(out=meanT_out[b], in_=meanT[:])
                nc.sync.dma_start(out=logvarT_out[b], in_=logvT[:])
                nc.sync.dma_start(out=zT_out[b], in_=zcT[:LAT, cols])
                xw = psB.tile([128, 64], dt.float32, tag="xwn")
                nc.tensor.matmul(out=xw[:], lhsT=zcT[:, cols], rhs=w4[:],
                                 start=True, stop=True)
                xws = sbufp.tile([128, 64], dt.float32, tag="xws")
                nc.scalar.activation(out=xws[:], in_=xw[:], func=AF.Copy,
                                     scale=dinvsb[:, b:b + 1])
                ship_table(4, b, xws)

            def post4(b, aggd):
                hdT = sbufp.tile([64, 128], dt.float32, tag="hdT")
                nc.scalar.activation(out=hdT[:], in_=aggd[:], func=AF.Relu,
                                     bias=b4[:])
                xw = psB.tile([128, 64], dt.float32, tag="xwn")
                nc.tensor.matmul(out=xw[:], lhsT=hdT[:], rhs=w5[:],
                                 start=True, stop=True)
                xws = sbufp.tile([128, 64], dt.float32, tag="xws")
                nc.scalar.activation(out=xws[:], in_=xw[:], func=AF.Copy,
                                     scale=dinvsb[:, b:b + 1])
                ship_table(5, b, xws)

            def post5(b, aggd):
                oT = sbufp.tile([64, 128], dt.float32, tag="oT")
                nc.scalar.activation(out=oT[:], in_=aggd[:], func=AF.Identity,
                                     bias=b5[:])
                nc.sync.dma_start(out=outT_out[b], in_=oT[:])

            import concourse.mybir as _mybir
            AF = _mybir.ActivationFunctionType
            propagate(1, post1)
            propagate(2, post23)
            propagate(4, post4)
            propagate(5, post5)

    nc.compile()
    return nc


def _prep_all(feature, condition, edge_index,
              enc_i2h_W, enc_i2h_b, enc_mean_W, enc_mean_b,
              enc_logvar_W, enc_logvar_b,
              dec_i2h_W, dec_i2h_b, dec_out_W, dec_out_b):
    src = np.asarray(edge_index[0], np.int64)
    dst = np.asarray(edge_index[1], np.int64)
    dinv, T_LO, T_HI, cores_edges = _prep_edges(src, dst)

    # noise identical to reference: jax threefry with key 42
    import jax
    import jax.numpy as jnp
    cpu = jax.local_devices(backend="cpu")[0]
    with jax.default_device(cpu):
        noise = np.asarray(jax.random.normal(jax.random.key(42), (N, LAT),
                                             jnp.float32))

    feature = np.asarray(feature, np.float32)
    condition = np.asarray(condition, np.float32)
    x = np.concatenate([feature, condition], axis=1)          # [N, 96]

    w1 = np.asarray(enc_i2h_W, np.float32)
    w23 = np.concatenate([np.asarray(enc_mean_W, np.float32),
                          np.asarray(enc_logvar_W, np.float32)], axis=1)
    w4 = np.asarray(dec_i2h_W, np.float32)
    w5 = np.asarray(dec_out_W, np.float32)
    b1 = np.asarray(enc_i2h_b, np.float32).reshape(64, 1)
    b23 = np.concatenate([np.asarray(enc_mean_b, np.float32),
                          np.asarray(enc_logvar_b, np.float32)]).reshape(64, 1)
    b4 = np.asarray(dec_i2h_b, np.float32).reshape(64, 1)
    b5 = np.asarray(dec_out_b, np.float32).reshape(64, 1)
    iota = np.broadcast_to(np.arange(128, dtype=np.float32), (128, 128)).copy()

    in_maps = []
    for c in range(NCORES):
        lo_n, hi_n = c * SH, (c + 1) * SH
        ilo, ihi, dmo, dmh = cores_edges[c]
        xTs = np.zeros((96, NSH), np.float32)
        xTs[:, :SH] = x[lo_n:hi_n].T
        condT = np.zeros((64, NSH), np.float32)
        condT[LAT:64, :SH] = condition[lo_n:hi_n].T
        noiseT = np.zeros((LAT, NSH), np.float32)
        noiseT[:, :SH] = noise[lo_n:hi_n].T
        dsb = np.ones((128, NB), np.float32)
        dv = np.ones(NSH, np.float32)
        dv[:SH] = dinv[lo_n:hi_n]
        dsb[:, :] = dv.reshape(NB, BLK).T
        drep = np.broadcast_to(dv, (64, NSH)).copy()
        in_maps.append({
            "xT_in": xTs, "condT_in": condT, "noiseT_in": noiseT,
            "dinvsb_in": dsb, "dinvrep_in": drep, "iota_in": iota,
            "idxlo_in": ilo, "idxhi_in": ihi, "dmlo_in": dmo, "dmhi_in": dmh,
            "w1_in": w1, "w23_in": w23, "w4_in": w4, "w5_in": w5,
            "b1_in": b1, "b23_in": b23, "b4_in": b4, "b5_in": b5,
        })
    return T_LO, T_HI, in_maps


def kernel(**inputs):
    from concourse.bass_utils import run_bass_kernel_spmd

    T_LO, T_HI, in_maps = _prep_all(**inputs)

    key = hashlib.sha1(T_LO.tobytes() + T_HI.tobytes()).hexdigest()
    if key not in _cache:
        _cache[key] = _build_program(T_LO, T_HI)
    nc = _cache[key]

    r = run_bass_kernel_spmd(nc, in_maps, core_ids=list(range(NCORES)),
                             trace=False)

    def asm(name, d):
        parts = []
        for c in range(NCORES):
            a = r.results[c][name]            # [NB, d, 128]
            a = a.transpose(0, 2, 1).reshape(NSH, d)[:SH]
            parts.append(a)
        return np.ascontiguousarray(np.concatenate(parts, axis=0))

    z = asm("zT_out", LAT)
    mean = asm("meanT_out", LAT)
    logvar = asm("logvarT_out", LAT)
    out = asm("outT_out", FEAT)
    return (z, mean, logvar, out)


# revision 9
# speedup vs baseline: 22.8962x; 22.8962x over previous
"""Trainium2 Bass kernel for CombinedHiddenGCVAE (GCN conditional VAE).

Graph: N=50000 nodes, E=800000 edges. Five GCN propagations (enc_i2h,
enc_mean+enc_logvar fused, dec_i2h, dec_out) over the same normalized
adjacency D^-1/2 (A+I) D^-1/2.

Distribution: nodes sharded 8 ways (6250/core, padded to 49 blocks of 128).
Each conv k: every core builds its shard of the "gather table"
xw_k[n] = dinv[n] * (h_{k-1} @ W_k)[n], AllGathers it into a full
[50176, 64] DRAM table, then processes its own dst-edges: dma_gather the
src rows, segmented-sum by dst via one-hot S-matrix matmuls accumulated in
PSUM per 128-dst block, scale columns by dinv[dst], bias/activation, and
immediately compute + ship the next conv's table rows.
"""
import hashlib
import numpy as np

N = 50000
E = 800000
FEAT, COND, HID, LAT = 64, 32, 64, 32
NCORES = 8
SH = 6250                  # real nodes per core
BLK = 128
NB = 49                    # blocks per core (49*128 = 6272)
NSH = NB * BLK             # padded shard rows
QCH = 7                    # blocks per AllGather chunk (7 chunks of 7)
NQ = NB // QCH             # 7 AG chunks
ROWS_Q = NCORES * QCH * BLK   # 7168 table rows per AG chunk
V = NSH * NCORES           # 50176 table rows
HALFV = V // 2             # 25088 (int16 gather split)
SG = 4                     # blocks per gather supergroup
NSG = (NB + SG - 1) // SG  # 13
SCHUNK = 8                 # tiles per S-build op

_cache = {}


def _trow(n):
    """node id -> table row (AllGather chunk-major layout)."""
    n = np.asarray(n)
    c, r = np.divmod(n, SH)
    b, p = np.divmod(r, BLK)
    q, bq = np.divmod(b, QCH)
    return q * ROWS_Q + c * (QCH * BLK) + bq * BLK + p


def _wrap16_cols(idx_tile):
    """[128] idx of one tile -> [128, 8] int16 wrapped (16-partition, replicated)."""
    a = idx_tile.astype(np.int16).reshape(8, 16).T        # [16, 8]
    return np.tile(a, (8, 1))                             # [128, 8]


def _prep_edges(src, dst):
    """Host-side per-core edge layout. Returns per-core tensors + shared tile
    counts."""
    deg = np.bincount(dst, minlength=N).astype(np.float32) + 1.0
    dinv = (1.0 / np.sqrt(deg)).astype(np.float32)

    per_core = []
    for c in range(NCORES):
        lo_n, hi_n = c * SH, (c + 1) * SH
        m = (dst >= lo_n) & (dst < hi_n)
        s = np.concatenate([src[m], np.arange(lo_n, hi_n, dtype=np.int64)])
        dl = np.concatenate([dst[m] - lo_n, np.arange(SH, dtype=np.int64)])
        tr = _trow(s)
        b = dl // BLK
        dm = (dl % BLK).astype(np.float32)
        hi = tr >= HALFV
        per_core.append((tr, b, dm, hi))

    # shared tile counts per (block, half)
    T_LO = np.zeros(NB, np.int64)
    T_HI = np.zeros(NB, np.int64)
    for (tr, b, dm, hi) in per_core:
        for half, T in ((False, T_LO), (True, T_HI)):
            sel = b[hi == half]
            cnt = np.bincount(sel, minlength=NB)
            T = np.maximum(T, (cnt + BLK - 1) // BLK, out=T)
    T_LO = np.maximum(T_LO, 1)
    T_HI = np.maximum(T_HI, 1)

    TT_LO, TT_HI = int(T_LO.sum()), int(T_HI.sum())

    # build padded per-core streams
    cores = []
    for (tr, b, dm, hi) in per_core:
        idx_cols = {"lo": np.zeros((128, TT_LO * 8), np.int16),
                    "hi": np.zeros((128, TT_HI * 8), np.int16)}
        dmod = {"lo": np.full((128, TT_LO), 999.0, np.float32),
                "hi": np.full((128, TT_HI), 999.0, np.float32)}
        off = {"lo": 0, "hi": 0}
        for blk in range(NB):
            for half, key, T in ((False, "lo", T_LO), (True, "hi", T_HI)):
                sel = (b == blk) & (hi == half)
                rows = tr[sel] - (HALFV if half else 0)
                dms = dm[sel]
                ntile = int(T[blk])
                pad = ntile * BLK - len(rows)
                rows = np.concatenate([rows, np.zeros(pad, np.int64)])
                dmsp = np.concatenate([dms, np.full(pad, 999.0, np.float32)])
                o = off[key]
                for t in range(ntile):
                    idx_cols[key][:, (o + t) * 8:(o + t + 1) * 8] = \
                        _wrap16_cols(rows[t * BLK:(t + 1) * BLK])
                    dmod[key][:, o + t] = dmsp[t * BLK:(t + 1) * BLK]
                off[key] += ntile
        cores.append((idx_cols["lo"], idx_cols["hi"], dmod["lo"], dmod["hi"]))
    return dinv, T_LO, T_HI, cores


def _build_program(T_LO, T_HI):
    import concourse.bass as bass
    import concourse.bacc as bacc
    import concourse.mybir as mybir
    import concourse.tile as tile

    dt = mybir.dt
    TT_LO, TT_HI = int(T_LO.sum()), int(T_HI.sum())
    # supergroup tile spans per stream
    sg_blocks = [list(range(s * SG, min((s + 1) * SG, NB))) for s in range(NSG)]
    LO_OFF = np.concatenate([[0], np.cumsum(T_LO)]).astype(int)
    HI_OFF = np.concatenate([[0], np.cumsum(T_HI)]).astype(int)
    MAXSG_LO = max(int(T_LO[bs[0]:bs[-1] + 1].sum()) for bs in sg_blocks)
    MAXSG_HI = max(int(T_HI[bs[0]:bs[-1] + 1].sum()) for bs in sg_blocks)
    MAXT = int(max(T_LO.max(), T_HI.max()))

    nc = bacc.Bacc("TRN2", target_bir_lowering=False, debug=False,
                   num_devices=NCORES, dynamic_dma_scratch_size=49152)

    # ---- I/O ----
    xT_in = nc.dram_tensor("xT_in", [FEAT + COND, NSH], dt.float32,
                           kind="ExternalInput")
    condT_in = nc.dram_tensor("condT_in", [64, NSH], dt.float32,
                              kind="ExternalInput")   # rows 32:64 = cond^T
    noiseT_in = nc.dram_tensor("noiseT_in", [LAT, NSH], dt.float32,
                               kind="ExternalInput")
    dinvsb_in = nc.dram_tensor("dinvsb_in", [128, NB], dt.float32,
                               kind="ExternalInput")
    dinvrep_in = nc.dram_tensor("dinvrep_in", [64, NSH], dt.float32,
                                kind="ExternalInput")
    iota_in = nc.dram_tensor("iota_in", [128, 128], dt.float32,
                             kind="ExternalInput")
    idxlo_in = nc.dram_tensor("idxlo_in", [128, TT_LO * 8], dt.int16,
                              kind="ExternalInput")
    idxhi_in = nc.dram_tensor("idxhi_in", [128, TT_HI * 8], dt.int16,
                              kind="ExternalInput")
    dmlo_in = nc.dram_tensor("dmlo_in", [128, TT_LO], dt.float32,
                             kind="ExternalInput")
    dmhi_in = nc.dram_tensor("dmhi_in", [128, TT_HI], dt.float32,
                             kind="ExternalInput")
    w1_in = nc.dram_tensor("w1_in", [FEAT + COND, HID], dt.float32,
                           kind="ExternalInput")
    w23_in = nc.dram_tensor("w23_in", [HID, 64], dt.float32, kind="ExternalInput")
    w4_in = nc.dram_tensor("w4_in", [64, HID], dt.float32, kind="ExternalInput")
    w5_in = nc.dram_tensor("w5_in", [HID, FEAT], dt.float32, kind="ExternalInput")
    b1_in = nc.dram_tensor("b1_in", [64, 1], dt.float32, kind="ExternalInput")
    b23_in = nc.dram_tensor("b23_in", [64, 1], dt.float32, kind="ExternalInput")
    b4_in = nc.dram_tensor("b4_in", [64, 1], dt.float32, kind="ExternalInput")
    b5_in = nc.dram_tensor("b5_in", [64, 1], dt.float32, kind="ExternalInput")

    zT_out = nc.dram_tensor("zT_out", [NB, LAT, BLK], dt.float32,
                            kind="ExternalOutput")
    meanT_out = nc.dram_tensor("meanT_out", [NB, LAT, BLK], dt.float32,
                               kind="ExternalOutput")
    logvarT_out = nc.dram_tensor("logvarT_out", [NB, LAT, BLK], dt.float32,
                                 kind="ExternalOutput")
    outT_out = nc.dram_tensor("outT_out", [NB, FEAT, BLK], dt.float32,
                              kind="ExternalOutput")

    # ---- collective buffers ----
    tables = {}
    agins = {}
    for k in (1, 2, 4, 5):
        tables[k] = nc.dram_tensor(f"table{k}", [V, 64], dt.float32,
                                   kind="Internal", addr_space="Shared")
        agins[k] = [nc.dram_tensor(f"agin{k}_{q}", [QCH * BLK, 64], dt.float32,
                                   kind="Internal") for q in range(NQ)]

    with tile.TileContext(nc) as tc:
        with (
            tc.tile_pool(name="const", bufs=1) as constp,
            tc.tile_pool(name="stlo", bufs=2) as stlop,
            tc.tile_pool(name="sthi", bufs=2) as sthip,
            tc.tile_pool(name="sbuf", bufs=4) as sbufp,
            tc.tile_pool(name="smat", bufs=4) as smatp,
            tc.tile_pool(name="psA", bufs=3, space="PSUM") as psA,
            tc.tile_pool(name="psB", bufs=3, space="PSUM") as psB,
        ):
            # ---- constants to SBUF ----
            xT = constp.tile([FEAT + COND, NSH], dt.float32)
            zcT = constp.tile([64, NSH], dt.float32)
            noiseT = constp.tile([LAT, NSH], dt.float32)
            dinvsb = constp.tile([128, NB], dt.float32)
            dinvrep = constp.tile([64, NSH], dt.float32)
            iota = constp.tile([128, 128], dt.float32)
            idxlo = constp.tile([128, TT_LO * 8], dt.int16)
            idxhi = constp.tile([128, TT_HI * 8], dt.int16)
            dmlo = constp.tile([128, TT_LO], dt.float32)
            dmhi = constp.tile([128, TT_HI], dt.float32)
            w1 = constp.tile([FEAT + COND, HID], dt.float32)
            w23 = constp.tile([HID, 64], dt.float32)
            w4 = constp.tile([64, HID], dt.float32)
            w5 = constp.tile([HID, FEAT], dt.float32)
            b1 = constp.tile([64, 1], dt.float32)
            b23 = constp.tile([64, 1], dt.float32)
            b4 = constp.tile([64, 1], dt.float32)
            b5 = constp.tile([64, 1], dt.float32)
            for t, src_ in ((xT, xT_in), (zcT, condT_in), (noiseT, noiseT_in),
                            (dinvsb, dinvsb_in), (dinvrep, dinvrep_in),
                            (iota, iota_in), (idxlo, idxlo_in), (idxhi, idxhi_in),
                            (dmlo, dmlo_in), (dmhi, dmhi_in),
                            (w1, w1_in), (w23, w23_in), (w4, w4_in), (w5, w5_in),
                            (b1, b1_in), (b23, b23_in), (b4, b4_in), (b5, b5_in)):
                nc.sync.dma_start(out=t[:], in_=src_[:])

            def ship_table(k, b, rowsT_sbuf):
                """rowsT_sbuf [128,64] = table rows for block b (already
                dinv-scaled); DMA to agin, fire AG chunk when complete."""
                q, bq = divmod(b, QCH)
                nc.sync.dma_start(
                    out=agins[k][q][bq * BLK:(bq + 1) * BLK, :],
                    in_=rowsT_sbuf[:])
                if bq == QCH - 1:
                    nc.gpsimd.collective_compute(
                        "AllGather", mybir.AluOpType.bypass,
                        replica_groups=[list(range(NCORES))],
                        ins=[agins[k][q][:]],
                        outs=[tables[k][q * ROWS_Q:(q + 1) * ROWS_Q, :]],
                    )

            # ---- phase 1: build table1 = dinv * (x @ W1) ----
            for b in range(NB):
                xw = psB.tile([128, 64], dt.float32, tag="xwn")
                nc.tensor.matmul(out=xw[:], lhsT=xT[:, b * BLK:(b + 1) * BLK],
                                 rhs=w1[:], start=True, stop=True)
                xws = sbufp.tile([128, 64], dt.float32, tag="xws")
                nc.scalar.activation(out=xws[:], in_=xw[:],
                                     func=mybir.ActivationFunctionType.Copy,
                                     scale=dinvsb[:, b:b + 1])
                ship_table(1, b, xws)

            # ---- propagation convs ----
            GT = 8   # tiles per gather instruction (1024 descriptors)

            def propagate(k, postproc):
                """gather from tables[k], segmented-sum, call postproc(b, aggd)
                with aggd = [64,128] SBUF tile of dinv-scaled aggregation."""
                stage_tiles = {"lo": {}, "hi": {}}

                def get_stage(key, g):
                    if g in stage_tiles[key]:
                        return stage_tiles[key][g]
                    TT = TT_LO if key == "lo" else TT_HI
                    idxs = idxlo if key == "lo" else idxhi
                    pool = stlop if key == "lo" else sthip
                    src = tables[k][:] if key == "lo" else tables[k][HALFV:, :]
                    L = min(GT, TT - g * GT)
                    st = pool.tile([128, L, 64], dt.float32, tag="g" + key,
                                   padded_shape=[128, GT, 64])
                    nc.gpsimd.dma_gather(
                        out_ap=st[:], in_ap=src,
                        idxs_ap=idxs[:, g * GT * 8:(g * GT + L) * 8],
                        num_idxs=L * BLK, num_idxs_reg=L * BLK, elem_size=64)
                    stage_tiles[key][g] = st
                    return st

                for b in range(NB):
                    agg = psA.tile([64, 128], dt.float32, tag="agg")
                    ntl, nth = int(T_LO[b]), int(T_HI[b])
                    first = True
                    for key, nt, base, dm in (
                            ("lo", ntl, int(LO_OFF[b]), dmlo),
                            ("hi", nth, int(HI_OFF[b]), dmhi)):
                        sm = smatp.tile([128, MAXT * 128], dt.float32,
                                        tag="sm")
                        nc.vector.tensor_tensor(
                            out=sm[:, :nt * 128],
                            in0=dm[:, base:base + nt]
                                .to_broadcast([128, nt, 128]),
                            in1=iota[:, None, :].to_broadcast([128, nt, 128]),
                            op=mybir.AluOpType.is_equal,
                        )
                        for t in range(nt):
                            gi, go = divmod(base + t, GT)
                            stage = get_stage(key, gi)
                            last = (key == "hi") and (t == nth - 1)
                            nc.tensor.matmul(
                                out=agg[:],
                                lhsT=stage[:, go, :],
                                rhs=sm[:, t * 128:(t + 1) * 128],
                                start=first, stop=last)
                            first = False
                    aggd = sbufp.tile([64, 128], dt.float32, tag="aggd")
                    nc.vector.tensor_mul(
                        out=aggd[:], in0=agg[:],
                        in1=dinvrep[:, b * BLK:(b + 1) * BLK])
                    postproc(b, aggd)

            AF = None  # set below

            def post1(b, aggd):
                hT = sbufp.tile([64, 128], dt.float32, tag="hT")
                nc.scalar.activation(out=hT[:], in_=aggd[:], func=AF.Relu,
                                     bias=b1[:])
                xw = psB.tile([128, 64], dt.float32, tag="xwn")
                nc.tensor.matmul(out=xw[:], lhsT=hT[:], rhs=w23[:],
                                 start=True, stop=True)
                xws = sbufp.tile([128, 64], dt.float32, tag="xws")
                nc.scalar.activation(out=xws[:], in_=xw[:], func=AF.Copy,
                                     scale=dinvsb[:, b:b + 1])
                ship_table(2, b, xws)

            def post23(b, aggd):
                cols = slice(b * BLK, (b + 1) * BLK)
                meanT = sbufp.tile([LAT, 128], dt.float32, tag="meanT")
                logvT = sbufp.tile([LAT, 128], dt.float32, tag="logvT")
                nc.scalar.activation(out=meanT[:], in_=aggd[:LAT, :],
                                     func=AF.Identity, bias=b23[:LAT, :])
                nc.scalar.activation(out=logvT[:], in_=aggd[LAT:, :],
                                     func=AF.Identity, bias=b23[LAT:, :])
                expT = sbufp.tile([LAT, 128], dt.float32, tag="expT")
                nc.scalar.activation(out=expT[:], in_=logvT[:], func=AF.Exp,
                                     scale=0.5)
                # z = noise * exp + mean, written into zcT rows 0:32
                nc.vector.tensor_mul(out=expT[:], in0=expT[:],
                                     in1=noiseT[:, cols])
                nc.vector.tensor_add(out=zcT[:LAT, cols], in0=expT[:],
                                     in1=meanT[:])
                nc.sync.dma_start(out=meanT_out[b], in_=meanT[:])
                nc.sync.dma_start(out=logvarT_out[b], in_=logvT[:])
                nc.sync.dma_start(out=zT_out[b], in_=zcT[:LAT, cols])
                xw = psB.tile([128, 64], dt.float32, tag="xwn")
                nc.tensor.matmul(out=xw[:], lhsT=zcT[:, cols], rhs=w4[:],
                                 start=True, stop=True)
                xws = sbufp.tile([128, 64], dt.float32, tag="xws")
                nc.scalar.activation(out=xws[:], in_=xw[:], func=AF.Copy,
                                     scale=dinvsb[:, b:b + 1])
                ship_table(4, b, xws)

            def post4(b, aggd):
                hdT = sbufp.tile([64, 128], dt.float32, tag="hdT")
                nc.scalar.activation(out=hdT[:], in_=aggd[:], func=AF.Relu,
                                     bias=b4[:])
                xw = psB.tile([128, 64], dt.float32, tag="xwn")
                nc.tensor.matmul(out=xw[:], lhsT=hdT[:], rhs=w5[:],
                                 start=True, stop=True)
                xws = sbufp.tile([128, 64], dt.float32, tag="xws")
                nc.scalar.activation(out=xws[:], in_=xw[:], func=AF.Copy,
                                     scale=dinvsb[:, b:b + 1])
                ship_table(5, b, xws)

            def post5(b, aggd):
                oT = sbufp.tile([64, 128], dt.float32, tag="oT")
                nc.scalar.activation(out=oT[:], in_=aggd[:], func=AF.Identity,
                                     bias=b5[:])
                nc.sync.dma_start(out=outT_out[b], in_=oT[:])

            import concourse.mybir as _mybir
            AF = _mybir.ActivationFunctionType
            propagate(1, post1)
            propagate(2, post23)
            propagate(4, post4)
            propagate(5, post5)

    nc.compile()
    return nc


def _prep_all(feature, condition, edge_index,
              enc_i2h_W, enc_i2h_b, enc_mean_W, enc_mean_b,
              enc_logvar_W, enc_logvar_b,
              dec_i2h_W, dec_i2h_b, dec_out_W, dec_out_b):
    src = np.asarray(edge_index[0], np.int64)
    dst = np.asarray(edge_index[1], np.int64)
    dinv, T_LO, T_HI, cores_edges = _prep_edges(src, dst)

    # noise identical to reference: jax threefry with key 42
    import jax
    import jax.numpy as jnp
    cpu = jax.local_devices(backend="cpu")[0]
    with jax.default_device(cpu):
        noise = np.asarray(jax.random.normal(jax.random.key(42), (N, LAT),
                                             jnp.float32))

    feature = np.asarray(feature, np.float32)
    condition = np.asarray(condition, np.float32)
    x = np.concatenate([feature, condition], axis=1)          # [N, 96]

    w1 = np.asarray(enc_i2h_W, np.float32)
    w23 = np.concatenate([np.asarray(enc_mean_W, np.float32),
                          np.asarray(enc_logvar_W, np.float32)], axis=1)
    w4 = np.asarray(dec_i2h_W, np.float32)
    w5 = np.asarray(dec_out_W, np.float32)
    b1 = np.asarray(enc_i2h_b, np.float32).reshape(64, 1)
    b23 = np.concatenate([np.asarray(enc_mean_b, np.float32),
                          np.asarray(enc_logvar_b, np.float32)]).reshape(64, 1)
    b4 = np.asarray(dec_i2h_b, np.float32).reshape(64, 1)
    b5 = np.asarray(dec_out_b, np.float32).reshape(64, 1)
    iota = np.broadcast_to(np.arange(128, dtype=np.float32), (128, 128)).copy()

    in_maps = []
    for c in range(NCORES):
        lo_n, hi_n = c * SH, (c + 1) * SH
        ilo, ihi, dmo, dmh = cores_edges[c]
        xTs = np.zeros((96, NSH), np.float32)
        xTs[:, :SH] = x[lo_n:hi_n].T
        condT = np.zeros((64, NSH), np.float32)
        condT[LAT:64, :SH] = condition[lo_n:hi_n].T
        noiseT = np.zeros((LAT, NSH), np.float32)
        noiseT[:, :SH] = noise[lo_n:hi_n].T
        dsb = np.ones((128, NB), np.float32)
        dv = np.ones(NSH, np.float32)
        dv[:SH] = dinv[lo_n:hi_n]
        dsb[:, :] = dv.reshape(NB, BLK).T
        drep = np.broadcast_to(dv, (64, NSH)).copy()
        in_maps.append({
            "xT_in": xTs, "condT_in": condT, "noiseT_in": noiseT,
            "dinvsb_in": dsb, "dinvrep_in": drep, "iota_in": iota,
            "idxlo_in": ilo, "idxhi_in": ihi, "dmlo_in": dmo, "dmhi_in": dmh,
            "w1_in": w1, "w23_in": w23, "w4_in": w4, "w5_in": w5,
            "b1_in": b1, "b23_in": b23, "b4_in": b4, "b5_in": b5,
        })
    return T_LO, T_HI, in_maps


def kernel(**inputs):
    from concourse.bass_utils import run_bass_kernel_spmd

    T_LO, T_HI, in_maps = _prep_all(**inputs)

    key = hashlib.sha1(T_LO.tobytes() + T_HI.tobytes()).hexdigest()
    if key not in _cache:
        _cache[key] = _build_program(T_LO, T_HI)
    nc = _cache[key]

    r = run_bass_kernel_spmd(nc, in_maps, core_ids=list(range(NCORES)),
                             trace=False)

    def asm(name, d):
        parts = []
        for c in range(NCORES):
            a = r.results[c][name]            # [NB, d, 128]
            a = a.transpose(0, 2, 1).reshape(NSH, d)[:SH]
            parts.append(a)
        return np.ascontiguousarray(np.concatenate(parts, axis=0))

    z = asm("zT_out", LAT)
    mean = asm("meanT_out", LAT)
    logvar = asm("logvarT_out", LAT)
    out = asm("outT_out", FEAT)
    return (z, mean, logvar, out)
